# revision 19
# baseline (speedup 1.0000x reference)
"""Trainium2 Bass kernel for hierarchical (sibling-group) softmax over
hyperplane margins.

For x:(8,64,128,128), normals/offsets:(1024,64), sibmat block-diagonal with
32-wide sibling groups:

    logits[b,m,h,w] = <x[b,:,h,w], normals[m]> - <normals[m], offsets[m]>
    out = exp(logits) / (group_sum_32(exp(logits)) + 1e-15)

Sharding: data-parallel over batch, one batch element per NeuronCore (8
cores), no collectives.  Per core, m-chunks of 128 rows live on partitions
and pixels on the free axis.

Design (v5; v4 measured 298 us, PE-issue-bound: the per-chunk exp bias
forced FD=512 ACT instructions whose serial PSUM frees staggered the
row-packed mm1 pairs, and each stalled mm1 head-of-line-blocked the packed
mm2/mm3 chains behind it in the in-order PE queue):
  1. The hyperplane bias is folded into the group matmuls: the mm2/mm3
     indicator weights are exp(-<n_m, o_m>) instead of 1.0 (values 1+-2e-3,
     exact to fp16), so Z = sum_m w_m exp(<x, n_m>) and the broadcast
     carries w_m * (1/Z).  The ACT exp is then bias-free.
  2. mm1 row-packed 2-way: x and normals are duplicated onto partitions
     64-127; chunks 2p / 2p+1 run concurrently in disjoint row bands into
     ONE [128, 1024] PSUM pair-tile (one chunk per bank).  A single
     bias-free FD=1024 exp drains the pair-tile, so both banks free
     atomically and the next pair co-issues - the pairing survives steady
     state.
  3. Z for one superblock lives in ONE [128, 512] PSUM tile: rows =
     64*half + 32*block + group, cols = pixel-within-half-block.  The four
     (half, block) accumulation chains sit in four distinct column bands
     (tile_position col = 64*half+32*blk) and pack 4-way; emitted as
     4-MM packets interleaved between mm1 pair-batches so they absorb the
     PE's ACT-paced idle gaps without head-of-line risk.
  4. 1/Z = exp(-ln Z) on ACT, one FD=512 Ln + Exp per superblock.
  5. PE broadcasts w*(1/Z) onto each chunk's partitions via the [32, 128]
     weighted indicator (4 replicas at row bands 0/32/64/96) into
     [128, 1024] fp32 PSUM tiles; DVE tensor_tensor multiplies e * bc at
     FD=1024 from PSUM (PSUM operand caps TT at 1x; fewer/larger
     instructions).  The Z tile borrows a buf from this same PSUM pool
     during the unit-idle window, which is what lets everything fit in 8
     banks: 2x[128,1024] logits pair-tiles + 2x[128,1024] broadcast bufs.
  6. e lives in one chunk-major [128, 16384] tile per superblock (col =
     mc*2048 + pix), filled by the exp's 2-run output AP, so every
     downstream read (mm2 moving, multiply src) is contiguous and the
     512 KB output DMAs land directly in the reference (M, H, W) layout;
     the host upcasts fp16 -> fp32.

A post-pass splits multi-wait instructions (walrus's TRN2 codegen encodes
at most one semaphore wait per compute instruction).  fp16 keeps all
value ranges exact to ~1.8e-3 of the fp32 reference (guarded by
input-range checks that fall back to an exact host implementation).
"""

import numpy as np

B, D, H, W = 8, 64, 128, 128
M = 1024
GROUP = 32
PIX = H * W          # 16384 pixels per batch element
BLK = 1024           # pixels per block
SBW = 2048           # pixels per superblock (2 blocks)
NSB = PIX // SBW     # 8 superblocks
MC = 128             # m-chunk width (partition dim)
NCHUNK = M // MC     # 8
NCORES = 8
FMAX = 512           # max moving free dim into one PSUM bank (fp32 out)

_cache = {}


_WAIT_OK_OPCODES = {"Call"}


def _split_excess_waits(nc):
    """Walrus's TRN2 codegen (CoreV3GenImpl setupSyncWait) encodes at most
    one semaphore wait per compute instruction (Matmult, TensorTensor, ...);
    Tile can legitimately attach several (e.g. waits on two input DMAs).
    Move all but one wait onto EventSemaphore instructions inserted just
    before the instruction on the same engine — ordering is identical."""
    import concourse.mybir as mybir

    n_fixed = 0
    for f in nc.m.functions:
        for blk in f.blocks:
            out = []
            changed = False
            for inst in blk.instructions:
                si = inst.sync_info
                if (
                    si is not None
                    and len(si.on_wait) > 1
                    and inst.opcode not in _WAIT_OK_OPCODES
                ):
                    waits = list(si.on_wait)
                    for j, w in enumerate(waits[:-1]):
                        out.append(
                            mybir.InstEventSemaphore(
                                name=f"{inst.name}-wsplit{j}",
                                opcode="EventSemaphore",
                                engine=inst.engine,
                                sync_info=mybir.SyncInfo(
                                    on_wait=[w], on_update=[]
                                ),
                            )
                        )
                    inst.sync_info = mybir.SyncInfo(
                        on_wait=[waits[-1]], on_update=list(si.on_update)
                    )
                    changed = True
                    n_fixed += 1
                out.append(inst)
            if changed:
                blk.instructions = out
    return n_fixed


def _build_nc():
    import concourse.bass as bass
    import concourse.mybir as mybir
    import concourse.tile as tile

    f32 = mybir.dt.float32
    f16 = mybir.dt.float16
    nc = bass.Bass()

    # x duplicated onto partitions 64-127 for 2-way row-packed mm1.
    x_in = nc.declare_dram_parameter("x_bf", [2 * D, PIX], f16,
                                     isOutput=False)
    # normals.T duplicated the same way: rows 0-63 and 64-127 identical.
    w_in = nc.declare_dram_parameter("normals_bf", [2 * D, M], f16,
                                     isOutput=False)
    # gsum_w[:, mc*32:(mc+1)*32]: [128, 32] weighted indicator,
    # [p, r] = exp(-<n_m, o_m>) iff r == 4*mc + p//32 (m = mc*128+p), so
    # the Z accumulation applies the hyperplane bias exactly.
    g_in = nc.declare_dram_parameter("gsum_w", [MC, NCHUNK * 32], f16,
                                     isOutput=False)
    # gbc_w[32*q + r, mc*128 + p]: replica q of the [32, 128] weighted
    # indicator that broadcasts group row r onto chunk partitions with the
    # destination m's exp(-bias) weight; replicas let mm3's stationary
    # start at the same partition base as its moving operand.
    b_in = nc.declare_dram_parameter("gbc_w", [4 * 32, NCHUNK * MC], f16,
                                     isOutput=False)
    y_out = nc.declare_dram_parameter("y", [M, PIX], f16, isOutput=True)

    with tile.TileContext(nc) as tc:
        with (
            tc.tile_pool(name="const", bufs=1) as cpool,
            tc.tile_pool(name="xin", bufs=3) as xpool,
            tc.tile_pool(name="expv", bufs=3) as epool,
            tc.tile_pool(name="recv", bufs=2) as rpool,
            tc.tile_pool(name="lnzv", bufs=2) as lpool,
            tc.tile_pool(name="outv", bufs=4) as opool,
            tc.tile_pool(name="bcv", bufs=2) as bcpool,
            tc.tile_pool(name="psl", bufs=2, space="PSUM") as pslp,
            tc.tile_pool(name="psbz", bufs=2, space="PSUM") as psbp,
        ):
            w_sb = cpool.tile([2 * D, M], f16)
            # 4-way split: the first mm1 gates on this load.
            for q in range(4):
                nc.sync.dma_start(w_sb[:, q * 256:(q + 1) * 256],
                                  w_in[:, q * 256:(q + 1) * 256])
            g_sb = cpool.tile([MC, NCHUNK * 32], f16)
            nc.sync.dma_start(g_sb[:], g_in[:])
            b_sb = cpool.tile([4 * 32, NCHUNK * MC], f16)
            nc.sync.dma_start(b_sb[:], b_in[:])

            x_of = {}
            e_of = {}      # sb -> [128, 8*2048] all-chunk tile
            rec_of = {}
            psz_of = {}

            def fetch_x(sb):
                if sb in x_of or sb >= NSB:
                    return
                x_t = xpool.tile([2 * D, SBW], f16, tag="x_t", name="x_t")
                # 4 quarter-DMAs land on 4 HWDGE queues in parallel, and
                # each mm1 off-batch only waits for its own quarter.
                for q in range(4):
                    nc.sync.dma_start(
                        x_t[:, q * FMAX:(q + 1) * FMAX],
                        x_in[:, sb * SBW + q * FMAX:
                             sb * SBW + (q + 1) * FMAX],
                    )
                x_of[sb] = x_t

            def mm1_batch(sb, pair, off):
                """One pair-batch: chunks (2*pair, 2*pair+1) x 512 pixels.
                Two concurrent MMs (row bands 0-63 / 64-127) into one
                [128, 1024] PSUM pair-tile (one chunk per bank), drained
                by a single bias-free FD=1024 exp whose 2-run output AP
                drops each chunk's half into its own chunk-major region
                of the superblock e tile."""
                x_t = x_of[sb]
                if pair == 0 and off == 0:
                    e_of[sb] = epool.tile([MC, NCHUNK * SBW], f16,
                                          tag="e_t", name="e_t")
                e_t = e_of[sb]
                ps = pslp.tile([MC, 2 * FMAX], f32, tag="ps_l", name="ps_l")
                for half in range(2):
                    mc = 2 * pair + half
                    nc.tensor.matmul(
                        ps[:, half * FMAX:(half + 1) * FMAX],
                        w_sb[half * D:(half + 1) * D,
                             mc * MC:(mc + 1) * MC],
                        x_t[half * D:(half + 1) * D,
                            off * FMAX:(off + 1) * FMAX],
                        start=True, stop=True,
                        tile_position=(half * D, 0),
                    )
                e_ap = e_t[:].rearrange("p (m c) -> p m c", c=SBW)[
                    :, 2 * pair:2 * pair + 2, off * FMAX:(off + 1) * FMAX
                ]
                nc.scalar.activation(
                    e_ap,
                    ps[:].rearrange("p (o c) -> p o c", o=2),
                    mybir.ActivationFunctionType.Exp,
                )

            def e_slice(sb, mc, blk, half):
                """[128, 512] AP for chunk mc's pixels blk*1024+half*512
                onward (chunk-major layout: col = mc*2048 + pix)."""
                e_t = e_of[sb]
                base = mc * SBW + blk * BLK + half * FMAX
                return e_t[:, base:base + FMAX]

            def mm2_packet(sg, mc):
                """Z chain links for chunk mc (4 MMs, one per (half, blk)
                column band; consecutive mc packets give 4-deep chain
                ILP)."""
                if mc == 0:
                    # Borrows a broadcast buf (same tag); only the first
                    # 512 cols are used.  Tenure is disjoint from the
                    # mul units' ps_b allocations within each superblock.
                    # The final superblock's Z instead borrows a logits
                    # pair-tile (idle once the last mm1 is done) so its
                    # chains never queue behind the previous superblock's
                    # DVE-paced mm3s — that wait was a 20 us tail.
                    if sg == NSB - 1:
                        psz_of[sg] = pslp.tile([MC, 2 * FMAX], f32,
                                               tag="ps_l", name="ps_z")
                    else:
                        psz_of[sg] = psbp.tile([MC, 2 * FMAX], f32,
                                               tag="ps_b", name="ps_z")
                ps_z = psz_of[sg]
                for blk in range(2):
                    for half in range(2):
                        r0 = 64 * half + 32 * blk
                        nc.tensor.matmul(
                            ps_z[r0:r0 + 32, 0:FMAX],
                            g_sb[:, mc * 32:(mc + 1) * 32],
                            e_slice(sg, mc, blk, half),
                            start=(mc == 0), stop=(mc == NCHUNK - 1),
                            tile_position=(0, r0),
                            skip_group_check=True,
                        )

            def emit_recip(sg):
                # 1/Z = exp(-ln Z) on ACT (Ln and Exp share one table
                # set); one FD=512 instruction pair per superblock.
                rec = rpool.tile([4 * 32, FMAX], f16, tag="rec", name="rec")
                rec_of[sg] = rec
                ps_z = psz_of.pop(sg)
                lnz = lpool.tile([4 * 32, FMAX], f32, tag="lnz", name="lnz")
                nc.scalar.activation(
                    lnz[:], ps_z[:, 0:FMAX],
                    mybir.ActivationFunctionType.Ln,
                )
                with nc.allow_low_precision(
                    reason="fp16 rounding of 1/Z feeding the broadcast "
                    "matmul; well within output tolerance"
                ):
                    nc.scalar.activation(
                        rec[:], lnz[:],
                        mybir.ActivationFunctionType.Exp,
                        scale=-1.0,
                    )

            outq = []

            def mul_unit(sg, mc):
                """PE broadcasts w*(1/Z) onto the chunk's 128 partitions
                (fp32 PSUM, MMs in 4 distinct row bands across 2 bufs),
                then DVE multiplies e * bc at FD=1024 from PSUM with a
                2-run access pattern on e.  The output DMA is emitted two
                units later so its wait-on-mul is pre-satisfied and never
                head-blocks the Sync queue."""
                rec = rec_of[sg]
                e_t = e_of[sg]
                o_t = opool.tile([MC, SBW], f16, tag="o_t", name="o_t")
                # ~3/8 units run their multiply on the otherwise-idle
                # GPSIMD engine (fed by a fast DVE 2x PSUM->SBUF copy):
                # the DVE multiply stream is the end-to-end critical
                # resource, and this splits it ~16.1/13.3 us per
                # superblock between the two engines.
                offload = mc in (1, 4, 6)
                bc = None
                if offload:
                    bc = bcpool.tile([MC, SBW], f16, tag="bc", name="bc")
                for blk in range(2):
                    ps_b = psbp.tile([MC, 2 * FMAX], f32, tag="ps_b",
                                     name="ps_b")
                    for half in range(2):
                        r0 = 64 * half + 32 * blk
                        nc.tensor.matmul(
                            ps_b[:, half * FMAX:(half + 1) * FMAX],
                            b_sb[r0:r0 + 32, mc * MC:(mc + 1) * MC],
                            rec[r0:r0 + 32, :],
                            start=True, stop=True,
                            tile_position=(r0, 0),
                        )
                    base = mc * SBW + blk * BLK
                    if offload:
                        with nc.allow_low_precision(
                            reason="fp16 staging of the broadcast for the "
                            "GPSIMD multiply; same rounding as the direct "
                            "fp16 multiply path"
                        ):
                            nc.vector.tensor_copy(
                                bc[:, blk * BLK:(blk + 1) * BLK], ps_b[:]
                            )
                    else:
                        nc.vector.tensor_mul(
                            o_t[:, blk * BLK:(blk + 1) * BLK],
                            e_t[:, base:base + BLK],
                            ps_b[:],
                        )
                if offload:
                    nc.gpsimd.tensor_mul(
                        o_t[:],
                        e_t[:, mc * SBW:(mc + 1) * SBW],
                        bc[:],
                    )
                outq.append((sg, mc, o_t))
                while len(outq) > 2:
                    emit_out()

            def emit_out(split=False):
                sg, mc, o_t = outq.pop(0)
                n = 4 if split else 1
                step = SBW // n
                for q in range(n):
                    nc.sync.dma_start(
                        y_out[mc * MC:(mc + 1) * MC,
                              sg * SBW + q * step:
                              sg * SBW + (q + 1) * step],
                        o_t[:, q * step:(q + 1) * step],
                    )

            # --- static schedule -------------------------------------
            # Packets of PE chain work (mm2 links / mm3+mul units) are
            # pumped between mm1 pair-batches so they fill the PE's
            # ACT-paced idle gaps; each packet is <= ~4 MMs.
            packets = []

            def pump(n=1):
                for _ in range(min(n, len(packets))):
                    packets.pop(0)()

            def queue_sg_work(sg):
                for mc in range(NCHUNK):
                    packets.append(lambda sg=sg, mc=mc: mm2_packet(sg, mc))
                packets.append(lambda sg=sg: emit_recip(sg))
                for mc in range(NCHUNK):
                    packets.append(lambda sg=sg, mc=mc: mul_unit(sg, mc))

            fetch_x(0)
            fetch_x(1)
            for sb in range(NSB):
                fetch_x(sb + 2)
                if sb >= 1:
                    queue_sg_work(sb - 1)
                for pair in range(4):
                    for off in range(4):
                        mm1_batch(sb, pair, off)
                        # pair 0 pumps double so the previous superblock's
                        # Z chains + reciprocal complete ~25% into this
                        # superblock; its mul units then spread over the
                        # remaining 75%, keeping the DVE stream smooth.
                        pump(2 if pair == 0 else 1)
                    pump(1)
            queue_sg_work(NSB - 1)
            pump(len(packets))
            assert not packets
            while outq:
                # the final DMAs have nothing left to hide behind: split
                # them across queues so they drain 4x faster
                emit_out(split=True)

    _split_excess_waits(nc)
    return nc


def _prep_core_inputs(x, normals, offsets):
    f16 = np.float16
    bias = np.einsum("md,md->m", normals, offsets).astype(np.float64)
    wgt = np.exp(-bias)                             # fold bias into Z/bc
    w_bf = np.ascontiguousarray(normals.T).astype(f16)
    w_bf = np.concatenate([w_bf, w_bf], axis=0)     # rows 64-127 duplicate

    gid = np.arange(M) // GROUP                     # global group of each m
    gsum = np.zeros((MC, NCHUNK * 32), np.float64)
    for mc in range(NCHUNK):
        for p in range(MC):
            m = mc * MC + p
            r = gid[m] % 32                         # group-within-block row
            gsum[p, mc * 32 + r] = wgt[m]
    gsum = gsum.astype(f16)
    gbc = np.zeros((32, NCHUNK * MC), np.float64)
    for mc in range(NCHUNK):
        for p in range(MC):
            m = mc * MC + p
            r = gid[m] % 32
            gbc[r, mc * MC + p] = wgt[m]
    gbc = np.tile(gbc, (4, 1)).astype(f16)

    in_maps = []
    for b in range(NCORES):
        x_bf = np.ascontiguousarray(x[b].reshape(D, PIX)).astype(f16)
        x_bf = np.concatenate([x_bf, x_bf], axis=0)
        in_maps.append(
            {"x_bf": x_bf, "normals_bf": w_bf, "gsum_w": gsum,
             "gbc_w": gbc}
        )
    return in_maps


def _sibmat_is_expected(sibmat):
    gid = np.arange(M) // GROUP
    expected = (gid[:, None] == gid[None, :]).astype(np.float32)
    return sibmat.shape == (M, M) and np.array_equal(sibmat, expected)


def _numpy_fallback(x, normals, offsets, sibmat):
    # Straight fp32 transcription of the reference; only used if sibmat is
    # not the expected 32-wide block-diagonal matrix.
    bias = np.einsum("md,md->m", normals, offsets)
    out = np.empty((B, M, H, W), np.float32)
    for b in range(B):
        logits = np.einsum("dhw,md->mhw", x[b], normals) - bias[:, None, None]
        logits -= np.max(logits, axis=0, keepdims=True)
        e = np.exp(logits)
        z = np.einsum("mhw,nm->nhw", e, sibmat)
        out[b] = e / (z + 1e-15)
    return out


def kernel(x, normals, offsets, sibmat, steps=None, trace=False, **_ignored):
    from concourse.bass_utils import run_bass_kernel_spmd

    x = np.ascontiguousarray(np.asarray(x, dtype=np.float32))
    normals = np.ascontiguousarray(np.asarray(normals, dtype=np.float32))
    offsets = np.ascontiguousarray(np.asarray(offsets, dtype=np.float32))
    sibmat = np.ascontiguousarray(np.asarray(sibmat, dtype=np.float32))

    if (
        not _sibmat_is_expected(sibmat)
        or np.abs(normals).max() > 0.5
        or np.abs(x).max() > 12.0
        or np.abs(np.einsum("md,md->m", normals, offsets)).max() > 0.25
    ):
        # unexpected structure or value ranges outside the fp16-safe
        # envelope of the device kernel: compute exactly on host
        return _numpy_fallback(x, normals, offsets, sibmat)

    if "nc" not in _cache:
        _cache["nc"] = _build_nc()
    nc = _cache["nc"]

    in_maps = _prep_core_inputs(x, normals, offsets)
    res = run_bass_kernel_spmd(nc, in_maps, list(range(NCORES)), trace=trace)
    out = np.stack(
        [np.asarray(r["y"]).astype(np.float32).reshape(M, H, W)
         for r in res.results]
    )
    kernel.last_result = res
    return out


# revision 24
# speedup vs baseline: 1.0763x; 1.0763x over previous
"""Trainium2 Bass kernel for hierarchical (sibling-group) softmax over
hyperplane margins.

For x:(8,64,128,128), normals/offsets:(1024,64), sibmat block-diagonal with
32-wide sibling groups:

    logits[b,m,h,w] = <x[b,:,h,w], normals[m]> - <normals[m], offsets[m]>
    out = exp(logits) / (group_sum_32(exp(logits)) + 1e-15)

Sharding: data-parallel over batch, one batch element per NeuronCore (8
cores), no collectives.  Per core, m-chunks of 128 rows live on partitions
and pixels on the free axis.

Design (v5; v4 measured 298 us, PE-issue-bound: the per-chunk exp bias
forced FD=512 ACT instructions whose serial PSUM frees staggered the
row-packed mm1 pairs, and each stalled mm1 head-of-line-blocked the packed
mm2/mm3 chains behind it in the in-order PE queue):
  1. The hyperplane bias is folded into the group matmuls: the mm2/mm3
     indicator weights are exp(-<n_m, o_m>) instead of 1.0 (values 1+-2e-3,
     exact to fp16), so Z = sum_m w_m exp(<x, n_m>) and the broadcast
     carries w_m * (1/Z).  The ACT exp is then bias-free.
  2. mm1 row-packed 2-way: x and normals are duplicated onto partitions
     64-127; chunks 2p / 2p+1 run concurrently in disjoint row bands into
     ONE [128, 1024] PSUM pair-tile (one chunk per bank).  A single
     bias-free FD=1024 exp drains the pair-tile, so both banks free
     atomically and the next pair co-issues - the pairing survives steady
     state.
  3. Z for one superblock lives in ONE [128, 512] PSUM tile: rows =
     64*half + 32*block + group, cols = pixel-within-half-block.  The four
     (half, block) accumulation chains sit in four distinct column bands
     (tile_position col = 64*half+32*blk) and pack 4-way; emitted as
     4-MM packets interleaved between mm1 pair-batches so they absorb the
     PE's ACT-paced idle gaps without head-of-line risk.
  4. 1/Z = exp(-ln Z) on ACT, one FD=512 Ln + Exp per superblock.
  5. PE broadcasts w*(1/Z) onto each chunk's partitions via the [32, 128]
     weighted indicator (4 replicas at row bands 0/32/64/96) into
     [128, 1024] fp32 PSUM tiles; DVE tensor_tensor multiplies e * bc at
     FD=1024 from PSUM (PSUM operand caps TT at 1x; fewer/larger
     instructions).  The Z tile borrows a buf from this same PSUM pool
     during the unit-idle window, which is what lets everything fit in 8
     banks: 2x[128,1024] logits pair-tiles + 2x[128,1024] broadcast bufs.
  6. e lives in one chunk-major [128, 16384] tile per superblock (col =
     mc*2048 + pix), filled by the exp's 2-run output AP, so every
     downstream read (mm2 moving, multiply src) is contiguous and the
     512 KB output DMAs land directly in the reference (M, H, W) layout;
     the host upcasts fp16 -> fp32.

A post-pass splits multi-wait instructions (walrus's TRN2 codegen encodes
at most one semaphore wait per compute instruction).  fp16 keeps all
value ranges exact to ~1.8e-3 of the fp32 reference (guarded by
input-range checks that fall back to an exact host implementation).
"""

import numpy as np

B, D, H, W = 8, 64, 128, 128
M = 1024
GROUP = 32
PIX = H * W          # 16384 pixels per batch element
BLK = 1024           # pixels per block
SBW = 2048           # pixels per superblock (2 blocks)
NSB = PIX // SBW     # 8 superblocks
MC = 128             # m-chunk width (partition dim)
NCHUNK = M // MC     # 8
NCORES = 8
FMAX = 512           # max moving free dim into one PSUM bank (fp32 out)

_cache = {}


_WAIT_OK_OPCODES = {"Call"}


def _split_excess_waits(nc):
    """Walrus's TRN2 codegen (CoreV3GenImpl setupSyncWait) encodes at most
    one semaphore wait per compute instruction (Matmult, TensorTensor, ...);
    Tile can legitimately attach several (e.g. waits on two input DMAs).
    Move all but one wait onto EventSemaphore instructions inserted just
    before the instruction on the same engine — ordering is identical."""
    import concourse.mybir as mybir

    n_fixed = 0
    for f in nc.m.functions:
        for blk in f.blocks:
            out = []
            changed = False
            for inst in blk.instructions:
                si = inst.sync_info
                if (
                    si is not None
                    and len(si.on_wait) > 1
                    and inst.opcode not in _WAIT_OK_OPCODES
                ):
                    waits = list(si.on_wait)
                    for j, w in enumerate(waits[:-1]):
                        out.append(
                            mybir.InstEventSemaphore(
                                name=f"{inst.name}-wsplit{j}",
                                opcode="EventSemaphore",
                                engine=inst.engine,
                                sync_info=mybir.SyncInfo(
                                    on_wait=[w], on_update=[]
                                ),
                            )
                        )
                    inst.sync_info = mybir.SyncInfo(
                        on_wait=[waits[-1]], on_update=list(si.on_update)
                    )
                    changed = True
                    n_fixed += 1
                out.append(inst)
            if changed:
                blk.instructions = out
    return n_fixed


def _build_nc():
    import concourse.bass as bass
    import concourse.mybir as mybir
    import concourse.tile as tile

    f32 = mybir.dt.float32
    f16 = mybir.dt.float16
    nc = bass.Bass()

    # x duplicated onto partitions 64-127 for 2-way row-packed mm1.
    x_in = nc.declare_dram_parameter("x_bf", [2 * D, PIX], f16,
                                     isOutput=False)
    # normals.T duplicated the same way: rows 0-63 and 64-127 identical.
    w_in = nc.declare_dram_parameter("normals_bf", [2 * D, M], f16,
                                     isOutput=False)
    # gsum_w[:, mc*32:(mc+1)*32]: [128, 32] weighted indicator,
    # [p, r] = exp(-<n_m, o_m>) iff r == 4*mc + p//32 (m = mc*128+p), so
    # the Z accumulation applies the hyperplane bias exactly.
    g_in = nc.declare_dram_parameter("gsum_w", [MC, NCHUNK * 32], f16,
                                     isOutput=False)
    # gbc_w[32*q + r, mc*128 + p]: replica q of the [32, 128] weighted
    # indicator that broadcasts group row r onto chunk partitions with the
    # destination m's exp(-bias) weight; replicas let mm3's stationary
    # start at the same partition base as its moving operand.
    b_in = nc.declare_dram_parameter("gbc_w", [4 * 32, NCHUNK * MC], f16,
                                     isOutput=False)
    y_out = nc.declare_dram_parameter("y", [M, PIX], f16, isOutput=True)

    with tile.TileContext(nc) as tc:
        with (
            tc.tile_pool(name="const", bufs=1) as cpool,
            tc.tile_pool(name="xin", bufs=3) as xpool,
            tc.tile_pool(name="expv", bufs=3) as epool,
            tc.tile_pool(name="recv", bufs=2) as rpool,
            tc.tile_pool(name="lnzv", bufs=2) as lpool,
            tc.tile_pool(name="outv", bufs=4) as opool,
            tc.tile_pool(name="psl", bufs=2, space="PSUM") as pslp,
            tc.tile_pool(name="psbz", bufs=2, space="PSUM") as psbp,
        ):
            w_sb = cpool.tile([2 * D, M], f16)
            # 4-way split: the first mm1 gates on this load.
            for q in range(4):
                nc.sync.dma_start(w_sb[:, q * 256:(q + 1) * 256],
                                  w_in[:, q * 256:(q + 1) * 256])
            g_sb = cpool.tile([MC, NCHUNK * 32], f16)
            nc.sync.dma_start(g_sb[:], g_in[:])
            b_sb = cpool.tile([4 * 32, NCHUNK * MC], f16)
            nc.sync.dma_start(b_sb[:], b_in[:])

            x_of = {}
            e_of = {}      # sb -> [128, 8*2048] all-chunk tile
            rec_of = {}
            psz_of = {}

            def fetch_x(sb):
                if sb in x_of or sb >= NSB:
                    return
                x_t = xpool.tile([2 * D, SBW], f16, tag="x_t", name="x_t")
                # 4 quarter-DMAs land on 4 HWDGE queues in parallel, and
                # each mm1 off-batch only waits for its own quarter.
                for q in range(4):
                    nc.sync.dma_start(
                        x_t[:, q * FMAX:(q + 1) * FMAX],
                        x_in[:, sb * SBW + q * FMAX:
                             sb * SBW + (q + 1) * FMAX],
                    )
                x_of[sb] = x_t

            def mm1_batch(sb, pair, off):
                """One pair-batch: chunks (2*pair, 2*pair+1) x 512 pixels.
                Two concurrent MMs (row bands 0-63 / 64-127) into one
                [128, 1024] PSUM pair-tile (one chunk per bank), drained
                by a single bias-free FD=1024 exp whose 2-run output AP
                drops each chunk's half into its own chunk-major region
                of the superblock e tile."""
                x_t = x_of[sb]
                if pair == 0 and off == 0:
                    e_of[sb] = epool.tile([MC, NCHUNK * SBW], f16,
                                          tag="e_t", name="e_t")
                e_t = e_of[sb]
                ps = pslp.tile([MC, 2 * FMAX], f32, tag="ps_l", name="ps_l")
                for half in range(2):
                    mc = 2 * pair + half
                    nc.tensor.matmul(
                        ps[:, half * FMAX:(half + 1) * FMAX],
                        w_sb[half * D:(half + 1) * D,
                             mc * MC:(mc + 1) * MC],
                        x_t[half * D:(half + 1) * D,
                            off * FMAX:(off + 1) * FMAX],
                        start=True, stop=True,
                        tile_position=(half * D, 0),
                    )
                e_ap = e_t[:].rearrange("p (m c) -> p m c", c=SBW)[
                    :, 2 * pair:2 * pair + 2, off * FMAX:(off + 1) * FMAX
                ]
                nc.scalar.activation(
                    e_ap,
                    ps[:].rearrange("p (o c) -> p o c", o=2),
                    mybir.ActivationFunctionType.Exp,
                )

            def e_slice(sb, mc, blk, half):
                """[128, 512] AP for chunk mc's pixels blk*1024+half*512
                onward (chunk-major layout: col = mc*2048 + pix)."""
                e_t = e_of[sb]
                base = mc * SBW + blk * BLK + half * FMAX
                return e_t[:, base:base + FMAX]

            def mm2_packet(sg, mc):
                """Z chain links for chunk mc (4 MMs, one per (half, blk)
                column band; consecutive mc packets give 4-deep chain
                ILP)."""
                if mc == 0:
                    # Borrows a broadcast buf (same tag); only the first
                    # 512 cols are used.  Tenure is disjoint from the
                    # mul units' ps_b allocations within each superblock.
                    # The final superblock's Z instead borrows a logits
                    # pair-tile (idle once the last mm1 is done) so its
                    # chains never queue behind the previous superblock's
                    # DVE-paced mm3s — that wait was a 20 us tail.
                    if sg == NSB - 1:
                        psz_of[sg] = pslp.tile([MC, 2 * FMAX], f32,
                                               tag="ps_l", name="ps_z")
                    else:
                        psz_of[sg] = psbp.tile([MC, 2 * FMAX], f32,
                                               tag="ps_b", name="ps_z")
                ps_z = psz_of[sg]
                for blk in range(2):
                    for half in range(2):
                        r0 = 64 * half + 32 * blk
                        nc.tensor.matmul(
                            ps_z[r0:r0 + 32, 0:FMAX],
                            g_sb[:, mc * 32:(mc + 1) * 32],
                            e_slice(sg, mc, blk, half),
                            start=(mc == 0), stop=(mc == NCHUNK - 1),
                            tile_position=(0, r0),
                            skip_group_check=True,
                        )

            def emit_recip(sg):
                # 1/Z = exp(-ln Z) on ACT (Ln and Exp share one table
                # set); one FD=512 instruction pair per superblock.
                rec = rpool.tile([4 * 32, FMAX], f16, tag="rec", name="rec")
                rec_of[sg] = rec
                ps_z = psz_of.pop(sg)
                lnz = lpool.tile([4 * 32, FMAX], f32, tag="lnz", name="lnz")
                nc.scalar.activation(
                    lnz[:], ps_z[:, 0:FMAX],
                    mybir.ActivationFunctionType.Ln,
                )
                with nc.allow_low_precision(
                    reason="fp16 rounding of 1/Z feeding the broadcast "
                    "matmul; well within output tolerance"
                ):
                    nc.scalar.activation(
                        rec[:], lnz[:],
                        mybir.ActivationFunctionType.Exp,
                        scale=-1.0,
                    )

            outq = []

            def mul_unit(sg, mc):
                """PE broadcasts w*(1/Z) onto the chunk's 128 partitions
                (fp32 PSUM, MMs in 4 distinct row bands across 2 bufs),
                then DVE multiplies e * bc at FD=1024 from PSUM with a
                2-run access pattern on e.  The output DMA is emitted two
                units later so its wait-on-mul is pre-satisfied and never
                head-blocks the Sync queue."""
                rec = rec_of[sg]
                e_t = e_of[sg]
                o_t = opool.tile([MC, SBW], f16, tag="o_t", name="o_t")
                for blk in range(2):
                    ps_b = psbp.tile([MC, 2 * FMAX], f32, tag="ps_b",
                                     name="ps_b")
                    for half in range(2):
                        r0 = 64 * half + 32 * blk
                        nc.tensor.matmul(
                            ps_b[:, half * FMAX:(half + 1) * FMAX],
                            b_sb[r0:r0 + 32, mc * MC:(mc + 1) * MC],
                            rec[r0:r0 + 32, :],
                            start=True, stop=True,
                            tile_position=(r0, 0),
                        )
                    base = mc * SBW + blk * BLK
                    nc.vector.tensor_mul(
                        o_t[:, blk * BLK:(blk + 1) * BLK],
                        e_t[:, base:base + BLK],
                        ps_b[:],
                    )
                outq.append((sg, mc, o_t))
                while len(outq) > 2:
                    emit_out()

            def emit_out(split=False):
                sg, mc, o_t = outq.pop(0)
                n = 4 if split else 1
                step = SBW // n
                for q in range(n):
                    nc.sync.dma_start(
                        y_out[mc * MC:(mc + 1) * MC,
                              sg * SBW + q * step:
                              sg * SBW + (q + 1) * step],
                        o_t[:, q * step:(q + 1) * step],
                    )

            # --- static schedule -------------------------------------
            # Packets of PE chain work (mm2 links / mm3+mul units) are
            # pumped between mm1 pair-batches so they fill the PE's
            # ACT-paced idle gaps; each packet is <= ~4 MMs.
            packets = []

            def pump(n=1):
                for _ in range(min(n, len(packets))):
                    packets.pop(0)()

            def queue_sg_work(sg, units_only=False):
                if not units_only:
                    for mc in range(NCHUNK):
                        packets.append(
                            lambda sg=sg, mc=mc: mm2_packet(sg, mc))
                    packets.append(lambda sg=sg: emit_recip(sg))
                for mc in range(NCHUNK):
                    packets.append(lambda sg=sg, mc=mc: mul_unit(sg, mc))

            fetch_x(0)
            fetch_x(1)
            for sb in range(NSB):
                fetch_x(sb + 2)
                if sb >= 1:
                    # sg0's Z chains + reciprocal already ran inline
                    # during superblock 0 (the broadcast PSUM bufs are
                    # idle before the first units), so its muls can
                    # start the moment superblock 1 begins.
                    queue_sg_work(sb - 1, units_only=(sb == 1))
                for pair in range(4):
                    for off in range(4):
                        mm1_batch(sb, pair, off)
                        if sb == 0:
                            # lag-one-pair inline Z chains for sg0
                            if pair >= 1 and off < 2:
                                mm2_packet(0, 2 * (pair - 1) + off)
                        else:
                            # front-load so the previous superblock's Z
                            # chains + reciprocal complete ~20% into this
                            # superblock; its mul units then spread over
                            # the remaining 80%, keeping the DVE stream
                            # smooth.
                            pump(3 if (sb >= 2 and pair == 0 and off < 3)
                                 else 1)
                    if sb > 0:
                        pump(1)
                if sb == 0:
                    mm2_packet(0, 6)
                    mm2_packet(0, 7)
                    emit_recip(0)
            queue_sg_work(NSB - 1)
            pump(len(packets))
            assert not packets
            while outq:
                # the final DMAs have nothing left to hide behind: split
                # them across queues so they drain 4x faster
                emit_out(split=True)

    _split_excess_waits(nc)
    return nc


def _prep_core_inputs(x, normals, offsets):
    f16 = np.float16
    bias = np.einsum("md,md->m", normals, offsets).astype(np.float64)
    wgt = np.exp(-bias)                             # fold bias into Z/bc
    w_bf = np.ascontiguousarray(normals.T).astype(f16)
    w_bf = np.concatenate([w_bf, w_bf], axis=0)     # rows 64-127 duplicate

    gid = np.arange(M) // GROUP                     # global group of each m
    gsum = np.zeros((MC, NCHUNK * 32), np.float64)
    for mc in range(NCHUNK):
        for p in range(MC):
            m = mc * MC + p
            r = gid[m] % 32                         # group-within-block row
            gsum[p, mc * 32 + r] = wgt[m]
    gsum = gsum.astype(f16)
    gbc = np.zeros((32, NCHUNK * MC), np.float64)
    for mc in range(NCHUNK):
        for p in range(MC):
            m = mc * MC + p
            r = gid[m] % 32
            gbc[r, mc * MC + p] = wgt[m]
    gbc = np.tile(gbc, (4, 1)).astype(f16)

    in_maps = []
    for b in range(NCORES):
        x_bf = np.ascontiguousarray(x[b].reshape(D, PIX)).astype(f16)
        x_bf = np.concatenate([x_bf, x_bf], axis=0)
        in_maps.append(
            {"x_bf": x_bf, "normals_bf": w_bf, "gsum_w": gsum,
             "gbc_w": gbc}
        )
    return in_maps


def _sibmat_is_expected(sibmat):
    gid = np.arange(M) // GROUP
    expected = (gid[:, None] == gid[None, :]).astype(np.float32)
    return sibmat.shape == (M, M) and np.array_equal(sibmat, expected)


def _numpy_fallback(x, normals, offsets, sibmat):
    # Straight fp32 transcription of the reference; only used if sibmat is
    # not the expected 32-wide block-diagonal matrix.
    bias = np.einsum("md,md->m", normals, offsets)
    out = np.empty((B, M, H, W), np.float32)
    for b in range(B):
        logits = np.einsum("dhw,md->mhw", x[b], normals) - bias[:, None, None]
        logits -= np.max(logits, axis=0, keepdims=True)
        e = np.exp(logits)
        z = np.einsum("mhw,nm->nhw", e, sibmat)
        out[b] = e / (z + 1e-15)
    return out


def kernel(x, normals, offsets, sibmat, steps=None, trace=False, **_ignored):
    from concourse.bass_utils import run_bass_kernel_spmd

    x = np.ascontiguousarray(np.asarray(x, dtype=np.float32))
    normals = np.ascontiguousarray(np.asarray(normals, dtype=np.float32))
    offsets = np.ascontiguousarray(np.asarray(offsets, dtype=np.float32))
    sibmat = np.ascontiguousarray(np.asarray(sibmat, dtype=np.float32))

    if (
        not _sibmat_is_expected(sibmat)
        or np.abs(normals).max() > 0.5
        or np.abs(x).max() > 12.0
        or np.abs(np.einsum("md,md->m", normals, offsets)).max() > 0.25
    ):
        # unexpected structure or value ranges outside the fp16-safe
        # envelope of the device kernel: compute exactly on host
        return _numpy_fallback(x, normals, offsets, sibmat)

    if "nc" not in _cache:
        _cache["nc"] = _build_nc()
    nc = _cache["nc"]

    in_maps = _prep_core_inputs(x, normals, offsets)
    res = run_bass_kernel_spmd(nc, in_maps, list(range(NCORES)), trace=trace)
    out = np.stack(
        [np.asarray(r["y"]).astype(np.float32).reshape(M, H, W)
         for r in res.results]
    )
    kernel.last_result = res
    return out
